# revision 7
# baseline (speedup 1.0000x reference)
"""Trainium2 Bass kernel for nn_GCN32Diff (GCN + DiffPool-to-1-cluster head).

Reference computation (per graph b):
    H  = relu((A @ X) @ W1 + b1)          # [N, HID]
    S  = softmax(A @ H @ Ws, axis=-1)     # [N, 1] -> softmax over size-1 axis == 1.0
    Xp = S^T H = sum_n H[n, :]            # [1, HID]
    h  = relu(Xp @ Wd + bd)               # [1, FC]
    out= softmax(h @ Wc + bc)             # [1, C]

Because S is softmax over a singleton axis it is identically 1, so the second
A-einsum (and Ws / node_indicator) never affect the output.  The kernel
computes  Xp = sum_n relu(A @ (X @ W1) + b1)  directly.

Sharding: data-parallel over the batch axis.  8 NeuronCores, 2 graphs each.
The dominant cost is streaming A (filtre, 256 MB fp32) from HBM once.

Per-core dataflow (per graph):
  stage 0: XW = X @ W1            (PE transposes of X tiles + small matmuls)
  stage 1: G^T = XW^T @ A^T       accumulated over the 16 m-chunks in PSUM.
           A tiles are cast to bf16 during the SWDGE DMA (fp32 can't use DMA
           transpose and fp32 matmul is 4x slower on the PE), transposed on
           the tensor engine, and contracted at full PE rate in bf16 with
           fp32 PSUM accumulation.  The final output is a saturated one-hot
           softmax with ~1e4 logit margins, so bf16 input rounding (~1e-3
           relative) cannot flip the result.
           relu+bias+row-sum happens in one ACT instruction via accum_out.
  stage 3: dense head + softmax on [1, 10].
"""

import numpy as np
from contextlib import ExitStack

import concourse.bass as bass  # noqa: F401  (engine types referenced via nc)
import concourse.bacc as bacc
import concourse.mybir as mybir
import concourse.tile as tile
from concourse import bass_utils
from concourse.masks import make_identity

N_CORES = 8
B, N, F = 16, 2048, 128
HID, FC, C = 32, 512, 10
BC = B // N_CORES          # graphs per core
P = 128                    # partitions
NCH = N // P               # 16 node chunks of 128
QUAD = 512                 # nodes handled per strip
NQ = N // QUAD             # 4 strips per graph
CPQ = QUAD // P            # 4 chunks per strip

F32 = mybir.dt.float32
BF16 = mybir.dt.bfloat16
AFT = mybir.ActivationFunctionType
AX = mybir.AxisListType

# bf16 runs the big matmul path at full PE rate; fp32 is the bit-exact
# fallback (4x slower matmuls, 2x slower transposes, fp32 DMA).
BIG_MM_DT = BF16


def _build_program():
    nc = bacc.Bacc("TRN2", target_bir_lowering=False, debug=False,
                   num_devices=N_CORES)

    filtre_d = nc.dram_tensor("filtre", [BC, N, N], F32, kind="ExternalInput")
    x_d = nc.dram_tensor("x", [BC, N, F], F32, kind="ExternalInput")
    w1_d = nc.dram_tensor("w1", [F, HID], F32, kind="ExternalInput")
    b1_d = nc.dram_tensor("b1", [HID], F32, kind="ExternalInput")
    wd_d = nc.dram_tensor("wd", [HID, FC], F32, kind="ExternalInput")
    bd_d = nc.dram_tensor("bd", [FC], F32, kind="ExternalInput")
    wc_d = nc.dram_tensor("wc", [FC, C], F32, kind="ExternalInput")
    bc_d = nc.dram_tensor("bc", [C], F32, kind="ExternalInput")
    out_d = nc.dram_tensor("out", [BC, 1, C], F32, kind="ExternalOutput")

    with tile.TileContext(nc) as tc:
        with ExitStack() as ctx:
            _emit(ctx, tc, filtre_d, x_d, w1_d, b1_d, wd_d, bd_d, wc_d, bc_d,
                  out_d)
    nc.compile()
    return nc


def _emit(ctx, tc, filtre_d, x_d, w1_d, b1_d, wd_d, bd_d, wc_d, bc_d, out_d):
    nc = tc.nc
    mm_dt = BIG_MM_DT

    const = ctx.enter_context(tc.tile_pool(name="const", bufs=1))
    x_pool = ctx.enter_context(tc.tile_pool(name="x", bufs=2))
    xw_pool = ctx.enter_context(tc.tile_pool(name="xw", bufs=2))
    a_pool = ctx.enter_context(tc.tile_pool(name="a", bufs=3))
    at_pool = ctx.enter_context(tc.tile_pool(name="at", bufs=3))
    hx_pool = ctx.enter_context(tc.tile_pool(name="hx", bufs=2))
    ps_small = ctx.enter_context(tc.tile_pool(name="pss", bufs=3, space="PSUM"))
    ps_acc = ctx.enter_context(tc.tile_pool(name="psa", bufs=2, space="PSUM"))

    # ---- constants / weights ----
    w1_sb = const.tile([F, HID], F32)
    nc.sync.dma_start(out=w1_sb[:], in_=w1_d[:])
    b1_sb = const.tile([HID, 1], F32)
    nc.sync.dma_start(out=b1_sb[:], in_=b1_d[:].rearrange("(h one) -> h one", one=1))
    wd_sb = const.tile([HID, FC], F32)
    nc.sync.dma_start(out=wd_sb[:], in_=wd_d[:])
    bd_sb = const.tile([P, FC // P], F32)
    nc.sync.dma_start(out=bd_sb[:], in_=bd_d[:].rearrange("(c p) -> p c", p=P))
    wc_sb = const.tile([P, FC // P, C], F32)
    nc.sync.dma_start(out=wc_sb[:], in_=wc_d[:].rearrange("(c p) n -> p c n", p=P))
    bc_sb = const.tile([1, C], F32)
    nc.sync.dma_start(out=bc_sb[:], in_=bc_d[:].rearrange("(one n) -> one n", one=1))
    ident = const.tile([P, P], F32)
    make_identity(nc, ident[:])
    ident_mm = const.tile([P, P], mm_dt)
    make_identity(nc, ident_mm[:])

    for g in range(BC):
        # ---- stage 0: XW = X @ W1, stored as 16 tiles [m=128, HID] ----
        x_sb = x_pool.tile([P, NCH, F], F32, tag="x")
        nc.sync.dma_start(out=x_sb[:], in_=x_d[g].rearrange("(t p) f -> p t f", p=P))
        xt_sb = x_pool.tile([P, NCH, P], F32, tag="xt")
        xw_sb = xw_pool.tile([P, NCH, HID], mm_dt, tag="xw")
        for t in range(NCH):
            xt_ps = ps_small.tile([P, P], F32, tag="ps")
            nc.tensor.transpose(xt_ps[:], x_sb[:, t, :], ident[:])
            nc.vector.tensor_copy(xt_sb[:, t, :], xt_ps[:])
        for t in range(NCH):
            xw_ps = ps_small.tile([P, HID], F32, tag="ps")
            nc.tensor.matmul(xw_ps[:], xt_sb[:, t, :], w1_sb[:],
                             start=True, stop=True)
            # fp32 -> mm_dt cast happens in this copy
            nc.vector.tensor_copy(xw_sb[:, t, :], xw_ps[:])

        # ---- stage 1: G^T = XW^T @ A^T per 512-node strip ----
        xp_q = hx_pool.tile([HID, NQ], F32, tag="xq")
        for q in range(NQ):
            a_sb = a_pool.tile([P, CPQ, N], mm_dt, tag="a")
            # SWDGE DMA casts fp32 -> mm_dt in the datapath
            nc.gpsimd.dma_start(
                out=a_sb[:],
                in_=filtre_d[g, q * QUAD:(q + 1) * QUAD, :].rearrange(
                    "(c p) m -> p c m", p=P),
            )
            ps_g = ps_acc.tile([HID, QUAD], F32, tag="acc")
            for mc in range(NCH):
                at_sb = at_pool.tile([P, QUAD], mm_dt, tag="at")
                for c in range(CPQ):
                    tp_ps = ps_small.tile([P, P], mm_dt, tag="ps")
                    nc.tensor.transpose(
                        tp_ps[:], a_sb[:, c, mc * P:(mc + 1) * P], ident_mm[:])
                    if c < 3:
                        nc.vector.tensor_copy(at_sb[:, c * P:(c + 1) * P], tp_ps[:])
                    else:
                        nc.scalar.copy(at_sb[:, c * P:(c + 1) * P], tp_ps[:])
                nc.tensor.matmul(
                    ps_g[:],
                    xw_sb[:, mc, :],
                    at_sb[:],
                    start=(mc == 0), stop=(mc == NCH - 1),
                )
            # relu(G + b1) and row-sum over this strip's 512 nodes in one op
            h_sb = hx_pool.tile([HID, QUAD], F32, tag="h")
            nc.scalar.activation(out=h_sb[:], in_=ps_g[:], func=AFT.Relu,
                                 bias=b1_sb[:], accum_out=xp_q[:, q:q + 1])

        xp_sb = hx_pool.tile([HID, 1], F32, tag="xp")
        nc.vector.reduce_sum(out=xp_sb[:], in_=xp_q[:], axis=AX.X)

        # ---- stage 3: dense head + softmax ----
        h1_sb = hx_pool.tile([P, FC // P], F32, tag="h1")
        for cb in range(FC // P):
            h1_ps = ps_small.tile([P, 1], F32, tag="ps")
            nc.tensor.matmul(h1_ps[:], wd_sb[:, cb * P:(cb + 1) * P], xp_sb[:],
                             start=True, stop=True)
            nc.scalar.activation(out=h1_sb[:, cb:cb + 1], in_=h1_ps[:],
                                 func=AFT.Relu, bias=bd_sb[:, cb:cb + 1])
        o_ps = ps_small.tile([1, C], F32, tag="ps")
        for cb in range(FC // P):
            nc.tensor.matmul(o_ps[:], h1_sb[:, cb:cb + 1], wc_sb[:, cb, :],
                             start=(cb == 0), stop=(cb == FC // P - 1))
        z_sb = hx_pool.tile([1, C], F32, tag="z")
        nc.vector.tensor_add(z_sb[:], o_ps[:], bc_sb[:])
        nm_sb = hx_pool.tile([1, 1], F32, tag="nm")
        nc.vector.reduce_max(out=nm_sb[:], in_=z_sb[:], axis=AX.X, negate=True)
        e_sb = hx_pool.tile([1, C], F32, tag="e")
        s_sb = hx_pool.tile([1, 1], F32, tag="s")
        nc.scalar.activation(out=e_sb[:], in_=z_sb[:], func=AFT.Exp,
                             bias=nm_sb[:], accum_out=s_sb[:])
        r_sb = hx_pool.tile([1, 1], F32, tag="r")
        nc.vector.reciprocal(r_sb[:], s_sb[:])
        o_sb = hx_pool.tile([1, C], F32, tag="o")
        nc.vector.tensor_scalar_mul(o_sb[:], e_sb[:], r_sb[:])
        nc.sync.dma_start(out=out_d[g, :, :], in_=o_sb[:])


_NC_CACHE = None


def _get_program():
    global _NC_CACHE
    if _NC_CACHE is None:
        _NC_CACHE = _build_program()
    return _NC_CACHE


def make_in_maps(filtre, X, W1, b1, Wd, bd, Wc, bc):
    filtre = np.ascontiguousarray(filtre, dtype=np.float32)
    X = np.ascontiguousarray(X, dtype=np.float32)
    in_maps = []
    for i in range(N_CORES):
        lo, hi = i * BC, (i + 1) * BC
        in_maps.append({
            "filtre": filtre[lo:hi],
            "x": X[lo:hi],
            "w1": np.ascontiguousarray(W1, dtype=np.float32),
            "b1": np.ascontiguousarray(b1, dtype=np.float32),
            "wd": np.ascontiguousarray(Wd, dtype=np.float32),
            "bd": np.ascontiguousarray(bd, dtype=np.float32),
            "wc": np.ascontiguousarray(Wc, dtype=np.float32),
            "bc": np.ascontiguousarray(bc, dtype=np.float32),
        })
    return in_maps


def run_sharded(in_maps, trace=False, **kwargs):
    nc = _get_program()
    return bass_utils.run_bass_kernel_spmd(nc, in_maps, list(range(N_CORES)),
                                           trace=trace, **kwargs)


def kernel(filtre, X, node_indicator=None, W1=None, b1=None, Ws=None,
           Wd=None, bd=None, Wc=None, bc=None):
    in_maps = make_in_maps(filtre, X, W1, b1, Wd, bd, Wc, bc)
    res = run_sharded(in_maps)
    out = np.concatenate([res.results[i]["out"] for i in range(N_CORES)],
                         axis=0)
    return out.astype(np.float32)


# revision 9
# speedup vs baseline: 13.6804x; 13.6804x over previous
"""Trainium2 Bass kernel for nn_GCN32Diff (GCN + DiffPool-to-1-cluster head).

Reference computation (per graph b):
    H  = relu((A @ X) @ W1 + b1)          # [N, HID]
    S  = softmax(A @ H @ Ws, axis=-1)     # [N, 1] -> softmax over size-1 axis == 1.0
    Xp = S^T H = sum_n H[n, :]            # [1, HID]
    h  = relu(Xp @ Wd + bd)               # [1, FC]
    out= softmax(h @ Wc + bc)             # [1, C]

Because S is softmax over a singleton axis it is identically 1, so the second
A-einsum (and Ws / node_indicator) never affect the output.  The kernel
computes  Xp = sum_n relu(A @ (X @ W1) + b1)  directly.

Sharding: data-parallel over the batch axis.  8 NeuronCores, 2 graphs each.
The dominant cost is streaming A (filtre, 256 MB fp32) from HBM once.

Per-core dataflow (per graph):
  stage 0: XW = X @ W1            (PE transposes of X tiles + small matmuls)
  stage 1: G^T = XW^T @ A^T       accumulated over the 16 m-chunks in PSUM.
           A tiles are cast to bf16 during the SWDGE DMA (fp32 can't use DMA
           transpose and fp32 matmul is 4x slower on the PE), transposed on
           the tensor engine, and contracted at full PE rate in bf16 with
           fp32 PSUM accumulation.  The final output is a saturated one-hot
           softmax with ~1e4 logit margins, so bf16 input rounding (~1e-3
           relative) cannot flip the result.
           relu+bias+row-sum happens in one ACT instruction via accum_out.
  stage 3: dense head + softmax on [1, 10].
"""

import numpy as np
from contextlib import ExitStack

import concourse.bass as bass  # noqa: F401  (engine types referenced via nc)
import concourse.bacc as bacc
import concourse.mybir as mybir
import concourse.tile as tile
from concourse import bass_utils
from concourse.masks import make_identity

N_CORES = 8
B, N, F = 16, 2048, 128
HID, FC, C = 32, 512, 10
BC = B // N_CORES          # graphs per core
P = 128                    # partitions
NCH = N // P               # 16 node chunks of 128
QUAD = 512                 # nodes handled per strip
NQ = N // QUAD             # 4 strips per graph
CPQ = QUAD // P            # 4 chunks per strip

F32 = mybir.dt.float32
BF16 = mybir.dt.bfloat16
AFT = mybir.ActivationFunctionType
AX = mybir.AxisListType

# bf16 runs the big matmul path at full PE rate; fp32 is the bit-exact
# fallback (4x slower matmuls, 2x slower transposes, fp32 DMA).
BIG_MM_DT = BF16

# Benchmark knob: emit the whole computation REPEAT times in one program so
# device time can be measured as the slope over repeats (dispatch overhead
# through the axon tunnel is milliseconds, far above the kernel itself).
REPEAT = 1


def _build_program():
    nc = bacc.Bacc("TRN2", target_bir_lowering=False, debug=False,
                   num_devices=N_CORES)

    filtre_d = nc.dram_tensor("filtre", [BC, N, N], F32, kind="ExternalInput")
    x_d = nc.dram_tensor("x", [BC, N, F], F32, kind="ExternalInput")
    w1_d = nc.dram_tensor("w1", [F, HID], F32, kind="ExternalInput")
    b1_d = nc.dram_tensor("b1", [HID], F32, kind="ExternalInput")
    wd_d = nc.dram_tensor("wd", [HID, FC], F32, kind="ExternalInput")
    bd_d = nc.dram_tensor("bd", [FC], F32, kind="ExternalInput")
    wc_d = nc.dram_tensor("wc", [FC, C], F32, kind="ExternalInput")
    bc_d = nc.dram_tensor("bc", [C], F32, kind="ExternalInput")
    out_d = nc.dram_tensor("out", [BC, 1, C], F32, kind="ExternalOutput")

    with tile.TileContext(nc) as tc:
        with ExitStack() as ctx:
            _emit(ctx, tc, filtre_d, x_d, w1_d, b1_d, wd_d, bd_d, wc_d, bc_d,
                  out_d)
    nc.compile()
    return nc


def _emit(ctx, tc, filtre_d, x_d, w1_d, b1_d, wd_d, bd_d, wc_d, bc_d, out_d):
    nc = tc.nc
    mm_dt = BIG_MM_DT

    const = ctx.enter_context(tc.tile_pool(name="const", bufs=1))
    x_pool = ctx.enter_context(tc.tile_pool(name="x", bufs=2))
    xw_pool = ctx.enter_context(tc.tile_pool(name="xw", bufs=2))
    a_pool = ctx.enter_context(tc.tile_pool(name="a", bufs=3))
    at_pool = ctx.enter_context(tc.tile_pool(name="at", bufs=3))
    hx_pool = ctx.enter_context(tc.tile_pool(name="hx", bufs=2))
    ps_small = ctx.enter_context(tc.tile_pool(name="pss", bufs=3, space="PSUM"))
    ps_acc = ctx.enter_context(tc.tile_pool(name="psa", bufs=2, space="PSUM"))

    # ---- constants / weights ----
    w1_sb = const.tile([F, HID], F32)
    nc.sync.dma_start(out=w1_sb[:], in_=w1_d[:])
    b1_sb = const.tile([HID, 1], F32)
    nc.sync.dma_start(out=b1_sb[:], in_=b1_d[:].rearrange("(h one) -> h one", one=1))
    wd_sb = const.tile([HID, FC], F32)
    nc.sync.dma_start(out=wd_sb[:], in_=wd_d[:])
    bd_sb = const.tile([P, FC // P], F32)
    nc.sync.dma_start(out=bd_sb[:], in_=bd_d[:].rearrange("(c p) -> p c", p=P))
    wc_sb = const.tile([P, FC // P, C], F32)
    nc.sync.dma_start(out=wc_sb[:], in_=wc_d[:].rearrange("(c p) n -> p c n", p=P))
    bc_sb = const.tile([1, C], F32)
    nc.sync.dma_start(out=bc_sb[:], in_=bc_d[:].rearrange("(one n) -> one n", one=1))
    ident = const.tile([P, P], F32)
    make_identity(nc, ident[:])
    ident_mm = const.tile([P, P], mm_dt)
    make_identity(nc, ident_mm[:])

    for g in [g for _ in range(REPEAT) for g in range(BC)]:
        # ---- stage 0: XW = X @ W1, stored as 16 tiles [m=128, HID] ----
        x_sb = x_pool.tile([P, NCH, F], F32, tag="x")
        nc.sync.dma_start(out=x_sb[:], in_=x_d[g].rearrange("(t p) f -> p t f", p=P))
        xt_sb = x_pool.tile([P, NCH, P], F32, tag="xt")
        xw_sb = xw_pool.tile([P, NCH, HID], mm_dt, tag="xw")
        for t in range(NCH):
            xt_ps = ps_small.tile([P, P], F32, tag="ps")
            nc.tensor.transpose(xt_ps[:], x_sb[:, t, :], ident[:])
            nc.vector.tensor_copy(xt_sb[:, t, :], xt_ps[:])
        for t in range(NCH):
            xw_ps = ps_small.tile([P, HID], F32, tag="ps")
            nc.tensor.matmul(xw_ps[:], xt_sb[:, t, :], w1_sb[:],
                             start=True, stop=True)
            # fp32 -> mm_dt cast happens in this copy
            nc.vector.tensor_copy(xw_sb[:, t, :], xw_ps[:])

        # ---- stage 1: G^T = XW^T @ A^T per 512-node strip ----
        xp_q = hx_pool.tile([HID, NQ], F32, tag="xq")
        for q in range(NQ):
            a_sb = a_pool.tile([P, CPQ, N], mm_dt, tag="a")
            # SWDGE DMA casts fp32 -> mm_dt in the datapath
            nc.gpsimd.dma_start(
                out=a_sb[:],
                in_=filtre_d[g, q * QUAD:(q + 1) * QUAD, :].rearrange(
                    "(c p) m -> p c m", p=P),
            )
            ps_g = ps_acc.tile([HID, QUAD], F32, tag="acc")
            for mc in range(NCH):
                at_sb = at_pool.tile([P, QUAD], mm_dt, tag="at")
                for c in range(CPQ):
                    tp_ps = ps_small.tile([P, P], mm_dt, tag="ps")
                    nc.tensor.transpose(
                        tp_ps[:], a_sb[:, c, mc * P:(mc + 1) * P], ident_mm[:])
                    if c < 3:
                        nc.vector.tensor_copy(at_sb[:, c * P:(c + 1) * P], tp_ps[:])
                    else:
                        nc.scalar.copy(at_sb[:, c * P:(c + 1) * P], tp_ps[:])
                nc.tensor.matmul(
                    ps_g[:],
                    xw_sb[:, mc, :],
                    at_sb[:],
                    start=(mc == 0), stop=(mc == NCH - 1),
                )
            # relu(G + b1) and row-sum over this strip's 512 nodes in one op
            h_sb = hx_pool.tile([HID, QUAD], F32, tag="h")
            nc.scalar.activation(out=h_sb[:], in_=ps_g[:], func=AFT.Relu,
                                 bias=b1_sb[:], accum_out=xp_q[:, q:q + 1])

        xp_sb = hx_pool.tile([HID, 1], F32, tag="xp")
        nc.vector.reduce_sum(out=xp_sb[:], in_=xp_q[:], axis=AX.X)

        # ---- stage 3: dense head + softmax ----
        h1_sb = hx_pool.tile([P, FC // P], F32, tag="h1")
        for cb in range(FC // P):
            h1_ps = ps_small.tile([P, 1], F32, tag="ps")
            nc.tensor.matmul(h1_ps[:], wd_sb[:, cb * P:(cb + 1) * P], xp_sb[:],
                             start=True, stop=True)
            nc.scalar.activation(out=h1_sb[:, cb:cb + 1], in_=h1_ps[:],
                                 func=AFT.Relu, bias=bd_sb[:, cb:cb + 1])
        o_ps = ps_small.tile([1, C], F32, tag="ps")
        for cb in range(FC // P):
            nc.tensor.matmul(o_ps[:], h1_sb[:, cb:cb + 1], wc_sb[:, cb, :],
                             start=(cb == 0), stop=(cb == FC // P - 1))
        z_sb = hx_pool.tile([1, C], F32, tag="z")
        nc.vector.tensor_add(z_sb[:], o_ps[:], bc_sb[:])
        nm_sb = hx_pool.tile([1, 1], F32, tag="nm")
        nc.vector.reduce_max(out=nm_sb[:], in_=z_sb[:], axis=AX.X, negate=True)
        e_sb = hx_pool.tile([1, C], F32, tag="e")
        s_sb = hx_pool.tile([1, 1], F32, tag="s")
        nc.scalar.activation(out=e_sb[:], in_=z_sb[:], func=AFT.Exp,
                             bias=nm_sb[:], accum_out=s_sb[:])
        r_sb = hx_pool.tile([1, 1], F32, tag="r")
        nc.vector.reciprocal(r_sb[:], s_sb[:])
        o_sb = hx_pool.tile([1, C], F32, tag="o")
        nc.vector.tensor_scalar_mul(o_sb[:], e_sb[:], r_sb[:])
        nc.sync.dma_start(out=out_d[g, :, :], in_=o_sb[:])


_NC_CACHE = None


def _get_program():
    global _NC_CACHE
    if _NC_CACHE is None:
        _NC_CACHE = _build_program()
    return _NC_CACHE


def make_in_maps(filtre, X, W1, b1, Wd, bd, Wc, bc):
    filtre = np.ascontiguousarray(filtre, dtype=np.float32)
    X = np.ascontiguousarray(X, dtype=np.float32)
    in_maps = []
    for i in range(N_CORES):
        lo, hi = i * BC, (i + 1) * BC
        in_maps.append({
            "filtre": filtre[lo:hi],
            "x": X[lo:hi],
            "w1": np.ascontiguousarray(W1, dtype=np.float32),
            "b1": np.ascontiguousarray(b1, dtype=np.float32),
            "wd": np.ascontiguousarray(Wd, dtype=np.float32),
            "bd": np.ascontiguousarray(bd, dtype=np.float32),
            "wc": np.ascontiguousarray(Wc, dtype=np.float32),
            "bc": np.ascontiguousarray(bc, dtype=np.float32),
        })
    return in_maps


def run_sharded(in_maps, trace=False, **kwargs):
    nc = _get_program()
    return bass_utils.run_bass_kernel_spmd(nc, in_maps, list(range(N_CORES)),
                                           trace=trace, **kwargs)


def kernel(filtre, X, node_indicator=None, W1=None, b1=None, Ws=None,
           Wd=None, bd=None, Wc=None, bc=None):
    in_maps = make_in_maps(filtre, X, W1, b1, Wd, bd, Wc, bc)
    res = run_sharded(in_maps)
    out = np.concatenate([res.results[i]["out"] for i in range(N_CORES)],
                         axis=0)
    return out.astype(np.float32)


# revision 11
# speedup vs baseline: 59.1101x; 4.3208x over previous
"""Trainium2 Bass kernel for nn_GCN32Diff (GCN + DiffPool-to-1-cluster head).

Reference computation (per graph b):
    H  = relu((A @ X) @ W1 + b1)          # [N, HID]
    S  = softmax(A @ H @ Ws, axis=-1)     # [N, 1] -> softmax over size-1 axis == 1.0
    Xp = S^T H = sum_n H[n, :]            # [1, HID]
    h  = relu(Xp @ Wd + bd)               # [1, FC]
    out= softmax(h @ Wc + bc)             # [1, C]

Because S is softmax over a singleton axis it is identically 1, so the second
A-einsum (and Ws / node_indicator) never affect the output.  The kernel
computes  Xp = sum_n relu(A @ (X @ W1) + b1)  directly.

Sharding: data-parallel over the batch axis.  8 NeuronCores, 2 graphs each.
The dominant cost is streaming A (filtre, 256 MB fp32) from HBM once.

Per-core dataflow (per graph):
  stage 0: XW = X @ W1            (PE transposes of X tiles + small matmuls)
  stage 1: G^T = XW^T @ A^T       accumulated over the 16 m-chunks in PSUM.
           A tiles are cast to bf16 during the SWDGE DMA (fp32 can't use DMA
           transpose and fp32 matmul is 4x slower on the PE), transposed on
           the tensor engine, and contracted at full PE rate in bf16 with
           fp32 PSUM accumulation.  The final output is a saturated one-hot
           softmax with ~1e4 logit margins, so bf16 input rounding (~1e-3
           relative) cannot flip the result.
           relu+bias+row-sum happens in one ACT instruction via accum_out.
  stage 3: dense head + softmax on [1, 10].
"""

import numpy as np
from contextlib import ExitStack

import concourse.bass as bass  # noqa: F401  (engine types referenced via nc)
import concourse.bacc as bacc
import concourse.mybir as mybir
import concourse.tile as tile
from concourse import bass_utils
from concourse.masks import make_identity

N_CORES = 8
B, N, F = 16, 2048, 128
HID, FC, C = 32, 512, 10
BC = B // N_CORES          # graphs per core
P = 128                    # partitions
NCH = N // P               # 16 node chunks of 128
QUAD = 512                 # nodes handled per strip
NQ = N // QUAD             # 4 strips per graph
CPQ = QUAD // P            # 4 chunks per strip

F32 = mybir.dt.float32
BF16 = mybir.dt.bfloat16
AFT = mybir.ActivationFunctionType
AX = mybir.AxisListType

# bf16 runs the big matmul path at full PE rate; fp32 is the bit-exact
# fallback (4x slower matmuls, 2x slower transposes, fp32 DMA).
BIG_MM_DT = BF16

# Benchmark knob: emit the whole computation REPEAT times in one program so
# device time can be measured as the slope over repeats (dispatch overhead
# through the axon tunnel is milliseconds, far above the kernel itself).
REPEAT = 1


def _build_program():
    nc = bacc.Bacc("TRN2", target_bir_lowering=False, debug=False,
                   num_devices=N_CORES)

    filtre_d = nc.dram_tensor("filtre", [BC, N, N], F32, kind="ExternalInput")
    x_d = nc.dram_tensor("x", [BC, N, F], F32, kind="ExternalInput")
    w1_d = nc.dram_tensor("w1", [F, HID], F32, kind="ExternalInput")
    b1_d = nc.dram_tensor("b1", [HID], F32, kind="ExternalInput")
    wd_d = nc.dram_tensor("wd", [HID, FC], F32, kind="ExternalInput")
    bd_d = nc.dram_tensor("bd", [FC], F32, kind="ExternalInput")
    wc_d = nc.dram_tensor("wc", [FC, C], F32, kind="ExternalInput")
    bc_d = nc.dram_tensor("bc", [C], F32, kind="ExternalInput")
    out_d = nc.dram_tensor("out", [BC, 1, C], F32, kind="ExternalOutput")

    with tile.TileContext(nc) as tc:
        with ExitStack() as ctx:
            _emit(ctx, tc, filtre_d, x_d, w1_d, b1_d, wd_d, bd_d, wc_d, bc_d,
                  out_d)
    nc.compile()
    return nc


def _emit(ctx, tc, filtre_d, x_d, w1_d, b1_d, wd_d, bd_d, wc_d, bc_d, out_d):
    nc = tc.nc
    mm_dt = BIG_MM_DT

    const = ctx.enter_context(tc.tile_pool(name="const", bufs=1))
    x_pool = ctx.enter_context(tc.tile_pool(name="x", bufs=2))
    xw_pool = ctx.enter_context(tc.tile_pool(name="xw", bufs=2))
    a_pool = ctx.enter_context(tc.tile_pool(name="a", bufs=3))
    at_pool = ctx.enter_context(tc.tile_pool(name="at", bufs=3))
    hx_pool = ctx.enter_context(tc.tile_pool(name="hx", bufs=2))
    ps_small = ctx.enter_context(tc.tile_pool(name="pss", bufs=2, space="PSUM"))
    ps_big = ctx.enter_context(tc.tile_pool(name="psb", bufs=3, space="PSUM"))
    ps_acc = ctx.enter_context(tc.tile_pool(name="psa", bufs=2, space="PSUM"))

    # ---- constants / weights ----
    w1_sb = const.tile([F, HID], F32)
    nc.sync.dma_start(out=w1_sb[:], in_=w1_d[:])
    b1_sb = const.tile([HID, 1], F32)
    nc.sync.dma_start(out=b1_sb[:], in_=b1_d[:].rearrange("(h one) -> h one", one=1))
    wd_sb = const.tile([HID, FC], F32)
    nc.sync.dma_start(out=wd_sb[:], in_=wd_d[:])
    bd_sb = const.tile([P, FC // P], F32)
    nc.sync.dma_start(out=bd_sb[:], in_=bd_d[:].rearrange("(c p) -> p c", p=P))
    wc_sb = const.tile([P, FC // P, C], F32)
    nc.sync.dma_start(out=wc_sb[:], in_=wc_d[:].rearrange("(c p) n -> p c n", p=P))
    bc_sb = const.tile([1, C], F32)
    nc.sync.dma_start(out=bc_sb[:], in_=bc_d[:].rearrange("(one n) -> one n", one=1))
    ident = const.tile([P, P], F32)
    make_identity(nc, ident[:])
    ident_mm = const.tile([P, P], mm_dt)
    make_identity(nc, ident_mm[:])

    for g in [g for _ in range(REPEAT) for g in range(BC)]:
        # ---- stage 0: XW = X @ W1, stored as 16 tiles [m=128, HID] ----
        x_sb = x_pool.tile([P, NCH, F], F32, tag="x")
        nc.sync.dma_start(out=x_sb[:], in_=x_d[g].rearrange("(t p) f -> p t f", p=P))
        xt_sb = x_pool.tile([P, NCH, P], F32, tag="xt")
        xw_sb = xw_pool.tile([P, NCH, HID], mm_dt, tag="xw")
        for t in range(NCH):
            xt_ps = ps_small.tile([P, P], F32, tag="ps")
            nc.tensor.transpose(xt_ps[:], x_sb[:, t, :], ident[:])
            nc.vector.tensor_copy(xt_sb[:, t, :], xt_ps[:])
        for t in range(NCH):
            xw_ps = ps_small.tile([P, HID], F32, tag="ps")
            nc.tensor.matmul(xw_ps[:], xt_sb[:, t, :], w1_sb[:],
                             start=True, stop=True)
            # fp32 -> mm_dt cast happens in this copy
            nc.vector.tensor_copy(xw_sb[:, t, :], xw_ps[:])

        # ---- stage 1: G^T = XW^T @ A^T per 512-node strip ----
        xp_q = hx_pool.tile([HID, NQ], F32, tag="xq")
        for q in range(NQ):
            a_sb = a_pool.tile([P, CPQ, N], mm_dt, tag="a")
            # SWDGE DMA casts fp32 -> mm_dt in the datapath
            nc.gpsimd.dma_start(
                out=a_sb[:],
                in_=filtre_d[g, q * QUAD:(q + 1) * QUAD, :].rearrange(
                    "(c p) m -> p c m", p=P),
            )
            ps_g = ps_acc.tile([HID, QUAD], F32, tag="acc")
            # 8 transposed [128, 128] bf16 tiles fill one 2 KB PSUM bank;
            # evicting them with a single wide copy amortizes the per-
            # instruction PSUM access latency (~174 ns) 8x.  Copies
            # alternate DVE / ACT to split the eviction bandwidth.
            for half in range(NCH // 2):
                bt_ps = ps_big.tile([P, 2, CPQ, P], mm_dt, tag="bt")
                for j in range(2):
                    mc = half * 2 + j
                    for c in range(CPQ):
                        nc.tensor.transpose(
                            bt_ps[:, j, c, :],
                            a_sb[:, c, mc * P:(mc + 1) * P], ident_mm[:])
                at_sb = at_pool.tile([P, 2, QUAD], mm_dt, tag="at")
                if half % 2 == 0:
                    nc.vector.tensor_copy(at_sb[:], bt_ps[:])
                else:
                    nc.scalar.copy(at_sb[:], bt_ps[:])
                for j in range(2):
                    mc = half * 2 + j
                    nc.tensor.matmul(
                        ps_g[:],
                        xw_sb[:, mc, :],
                        at_sb[:, j, :],
                        start=(mc == 0), stop=(mc == NCH - 1),
                    )
            # relu(G + b1) and row-sum over this strip's 512 nodes in one op
            h_sb = hx_pool.tile([HID, QUAD], F32, tag="h")
            nc.scalar.activation(out=h_sb[:], in_=ps_g[:], func=AFT.Relu,
                                 bias=b1_sb[:], accum_out=xp_q[:, q:q + 1])

        xp_sb = hx_pool.tile([HID, 1], F32, tag="xp")
        nc.vector.reduce_sum(out=xp_sb[:], in_=xp_q[:], axis=AX.X)

        # ---- stage 3: dense head + softmax ----
        h1_sb = hx_pool.tile([P, FC // P], F32, tag="h1")
        for cb in range(FC // P):
            h1_ps = ps_small.tile([P, 1], F32, tag="ps")
            nc.tensor.matmul(h1_ps[:], wd_sb[:, cb * P:(cb + 1) * P], xp_sb[:],
                             start=True, stop=True)
            nc.scalar.activation(out=h1_sb[:, cb:cb + 1], in_=h1_ps[:],
                                 func=AFT.Relu, bias=bd_sb[:, cb:cb + 1])
        o_ps = ps_small.tile([1, C], F32, tag="ps")
        for cb in range(FC // P):
            nc.tensor.matmul(o_ps[:], h1_sb[:, cb:cb + 1], wc_sb[:, cb, :],
                             start=(cb == 0), stop=(cb == FC // P - 1))
        z_sb = hx_pool.tile([1, C], F32, tag="z")
        nc.vector.tensor_add(z_sb[:], o_ps[:], bc_sb[:])
        nm_sb = hx_pool.tile([1, 1], F32, tag="nm")
        nc.vector.reduce_max(out=nm_sb[:], in_=z_sb[:], axis=AX.X, negate=True)
        e_sb = hx_pool.tile([1, C], F32, tag="e")
        s_sb = hx_pool.tile([1, 1], F32, tag="s")
        nc.scalar.activation(out=e_sb[:], in_=z_sb[:], func=AFT.Exp,
                             bias=nm_sb[:], accum_out=s_sb[:])
        r_sb = hx_pool.tile([1, 1], F32, tag="r")
        nc.vector.reciprocal(r_sb[:], s_sb[:])
        o_sb = hx_pool.tile([1, C], F32, tag="o")
        nc.vector.tensor_scalar_mul(o_sb[:], e_sb[:], r_sb[:])
        nc.sync.dma_start(out=out_d[g, :, :], in_=o_sb[:])


_NC_CACHE = None


def _get_program():
    global _NC_CACHE
    if _NC_CACHE is None:
        _NC_CACHE = _build_program()
    return _NC_CACHE


def make_in_maps(filtre, X, W1, b1, Wd, bd, Wc, bc):
    filtre = np.ascontiguousarray(filtre, dtype=np.float32)
    X = np.ascontiguousarray(X, dtype=np.float32)
    in_maps = []
    for i in range(N_CORES):
        lo, hi = i * BC, (i + 1) * BC
        in_maps.append({
            "filtre": filtre[lo:hi],
            "x": X[lo:hi],
            "w1": np.ascontiguousarray(W1, dtype=np.float32),
            "b1": np.ascontiguousarray(b1, dtype=np.float32),
            "wd": np.ascontiguousarray(Wd, dtype=np.float32),
            "bd": np.ascontiguousarray(bd, dtype=np.float32),
            "wc": np.ascontiguousarray(Wc, dtype=np.float32),
            "bc": np.ascontiguousarray(bc, dtype=np.float32),
        })
    return in_maps


def run_sharded(in_maps, trace=False, **kwargs):
    nc = _get_program()
    return bass_utils.run_bass_kernel_spmd(nc, in_maps, list(range(N_CORES)),
                                           trace=trace, **kwargs)


def kernel(filtre, X, node_indicator=None, W1=None, b1=None, Ws=None,
           Wd=None, bd=None, Wc=None, bc=None):
    in_maps = make_in_maps(filtre, X, W1, b1, Wd, bd, Wc, bc)
    res = run_sharded(in_maps)
    out = np.concatenate([res.results[i]["out"] for i in range(N_CORES)],
                         axis=0)
    return out.astype(np.float32)
